# revision 1
# baseline (speedup 1.0000x reference)
"""Trainium2 Bass kernel for GQA attention prefill (Mistral-style, RoPE, causal).

B=1, S=2048, DIM=4096, 32 Q heads / 8 KV heads, HD=128, rope theta 1e6.

Sharding: tensor-parallel over heads across 8 cores. Core i gets Q heads
4i..4i+3 and KV head i. x is replicated (pre-transposed + bf16-cast on host).
Each core computes its 4 heads' attention and a partial output projection
(contraction over its 512 input dims of wo); the host sums the 8 partials.

Per-core dataflow (all matmuls bf16 with fp32 PSUM accumulation):
  phase A (per 128-row s block):
    xT tiles [c,s] (lhsT) x wT [c, q|k|v] (rhs) -> psum [s, 768]
    rope applied in [s, d] layout via stride-2 APs (DVE), cast bf16
    PE-transpose q/k 128x128 blocks -> resident QT/KT [d, s]; V kept [s, d]
  phase B (per 512-col q chunk t, per head h):
    scores_T [k,q] = KT_tile.T @ QT  (one matmul per 128-k tile, no accum)
    P_T = exp(scale * scores_T) on ACT (no max subtraction: |scores| < ~15),
    diagonal blocks masked by precomputed 0/1 tiles (DVE)
    attn_T [d, q] accumulated via lhsT=V tiles; denom via lhsT=ones
    (every row of the denom psum equals the column sum of P_T)
    normalize on DVE (reciprocal + multiply) -> at [d', s] bf16
  o-proj: psum [s,512e] accumulated over the 4 heads, lhsT=at slices,
    rhs=woT [d', e]; evacuate fp32 and DMA to the partial output.
"""

import numpy as np
import ml_dtypes

S = 2048
DIM = 4096
HD = 128
N_CORES = 8
QH_PER_CORE = 4  # 512 q dims per core
DQ = QH_PER_CORE * HD  # 512
SCALE = 1.0 / float(np.sqrt(HD))
SB = S // 128  # 16 s blocks
CB = DIM // 128  # 32 contraction blocks
NT = S // 512  # 4 q chunks
ET = DIM // 512  # 8 e tiles

bf16 = ml_dtypes.bfloat16

_RUNNER = None


ALL_STAGES = frozenset({"proj", "rope", "tpose", "scores", "pv", "oproj"})

# debug knobs for perf isolation (set by bench scripts)
NO_EXP = False
NO_OUTDMA = False
PT_CONST = False


def _build(reps=None, stages=ALL_STAGES):
    import concourse.bass as bass
    import concourse.mybir as mybir
    import concourse.tile as tile
    from concourse import bacc
    from concourse.masks import make_identity
    from contextlib import nullcontext

    dt = mybir.dt
    Exp = mybir.ActivationFunctionType.Exp

    nc = bacc.Bacc(
        "TRN2", target_bir_lowering=False, debug=False, num_devices=N_CORES
    )

    xt_d = nc.dram_tensor("xt", [DIM, S], dt.bfloat16, kind="ExternalInput").ap()
    wt_d = nc.dram_tensor("wt", [DIM, 768], dt.bfloat16, kind="ExternalInput").ap()
    wot_d = nc.dram_tensor("wot", [DQ, DIM], dt.bfloat16, kind="ExternalInput").ap()
    csd_d = nc.dram_tensor("csd", [128, S], dt.float32, kind="ExternalInput").ap()
    snd_d = nc.dram_tensor("snd", [128, S], dt.float32, kind="ExternalInput").ap()
    mask_d = nc.dram_tensor("mask", [512, 512], dt.bfloat16, kind="ExternalInput").ap()
    out_d = nc.dram_tensor("out", [S, DIM], dt.float32, kind="ExternalOutput").ap()

    with tile.TileContext(nc) as tc:
        with tc.For_i(0, reps, 1) if reps else nullcontext(), tc.tile_pool(
            name="const", bufs=1
        ) as cp:
            mask_sb = cp.tile([128, 4, 512], dt.bfloat16)
            nc.sync.dma_start(out=mask_sb, in_=mask_d.rearrange("(m p) n -> p m n", p=128))
            ones_sb = cp.tile([128, 128], dt.float32)
            nc.vector.memset(ones_sb, 1.0)
            ptc_sb = cp.tile([128, 512], dt.bfloat16)
            nc.vector.memset(ptc_sb, 0.5)
            ident_sb = cp.tile([128, 128], dt.bfloat16)
            make_identity(nc, ident_sb)

            qt_sb = cp.tile([128, QH_PER_CORE, S], dt.bfloat16)  # [d, h, s]
            kt_sb = cp.tile([128, S], dt.bfloat16)  # [d, s]
            v_sb = cp.tile([128, SB, HD], dt.bfloat16)  # [s128, sb, d]

            # ---------------- phase A: projections + rope (direct QT) ---------
            # Weights are the stationary operand; psum comes out as [d, s]
            # (already transposed for attention). Q/K rows are host-permuted
            # per head into [even-pairs | odd-pairs] so rope works on
            # partition halves (inputs share a base; outputs may shift).
            # V is PE-transposed back to [s, d] (16 blocks).
            with (
                tc.tile_pool(name="pa", bufs=2) as pa,
                tc.tile_pool(name="pap", bufs=3, space="PSUM") as pap,
            ):
                wt_sb = pa.tile([128, CB, 768], dt.bfloat16, bufs=1)
                nc.sync.dma_start(
                    out=wt_sb, in_=wt_d.rearrange("(cb c) n -> c cb n", c=128)
                )
                csd_sb = pa.tile([128, S], dt.float32, bufs=1)
                nc.sync.dma_start(out=csd_sb, in_=csd_d)
                snd_sb = pa.tile([128, S], dt.float32, bufs=1)
                nc.sync.dma_start(out=snd_sb, in_=snd_d)

                def rope_evac(ps, dest, s0):
                    # dest[0:64]   = a*cos - b*sin
                    # dest[64:128] = a*sin + b*cos   (a=rows 0:64, b=rows 64:128)
                    cs = csd_sb[:, s0 : s0 + 512]
                    sn = snd_sb[:, s0 : s0 + 512]
                    t1 = pa.tile([128, 512], dt.float32, tag="t1")
                    t2 = pa.tile([128, 512], dt.float32, tag="t2")
                    nc.vector.tensor_mul(t1, ps, cs)
                    nc.vector.tensor_mul(t2[0:64, :], ps[64:128, :], sn[64:128, :])
                    nc.vector.tensor_mul(t2[64:128, :], ps[0:64, :], sn[0:64, :])
                    nc.vector.tensor_sub(dest[0:64, :], t1[0:64, :], t2[0:64, :])
                    nc.vector.tensor_add(dest[64:128, :], t1[64:128, :], t2[64:128, :])

                for sc in range(4):  # s chunks of 512
                    s0 = sc * 512
                    xt_sb = pa.tile([128, CB, 512], dt.bfloat16, tag="xt")
                    nc.sync.dma_start(
                        out=xt_sb,
                        in_=xt_d.rearrange("(cb c) s -> c cb s", c=128)[
                            :, :, s0 : s0 + 512
                        ],
                    )
                    for dtile in range(6):  # 4 Q heads, K, V
                        ps = pap.tile([128, 512], dt.float32, tag="proj")
                        for cb in range(CB if "proj" in stages else 0):
                            nc.tensor.matmul(
                                ps,
                                lhsT=wt_sb[:, cb, dtile * 128 : (dtile + 1) * 128],
                                rhs=xt_sb[:, cb, :],
                                start=(cb == 0),
                                stop=(cb == CB - 1),
                            )
                        if "rope" not in stages:
                            continue
                        if dtile < QH_PER_CORE:
                            rope_evac(ps, qt_sb[:, dtile, s0 : s0 + 512], s0)
                        elif dtile == QH_PER_CORE:
                            rope_evac(ps, kt_sb[:, s0 : s0 + 512], s0)
                        else:
                            vt_st = pa.tile([128, 512], dt.bfloat16, tag="vt")
                            nc.vector.tensor_copy(vt_st, ps)
                            for b in range(4 if "tpose" in stages else 0):
                                pst = pap.tile(
                                    [128, 128], dt.bfloat16, tag="tp", bufs=2
                                )
                                nc.tensor.transpose(
                                    pst, vt_st[:, b * 128 : (b + 1) * 128], ident_sb
                                )
                                nc.vector.tensor_copy(
                                    v_sb[:, sc * 4 + b, :], pst
                                )

            # ---------------- phase B: attention + output projection ----------
            # Software-pipelined: the o-projection for chunk t-1 is emitted
            # between the per-head attention groups of chunk t, so PE has
            # dense matmul work while ACT runs the exps of the current chunk.
            with (
                tc.tile_pool(name="pb", bufs=2) as pb,
                tc.tile_pool(name="pbp", bufs=2, space="PSUM") as pbp,
            ):
                woT_sb = pb.tile([128, QH_PER_CORE, DIM], dt.bfloat16, bufs=1)
                nc.sync.dma_start(
                    out=woT_sb, in_=wot_d.rearrange("(db p) e -> p db e", p=128)
                )

                def oproj_group(t, sbl, ats):
                    if "oproj" not in stages:
                        return
                    o_sb = pb.tile([128, DIM], dt.float32, tag="osb")
                    for e in range(ET):
                        ps_out = pbp.tile([128, 512], dt.float32, tag="oproj")
                        for h in range(QH_PER_CORE):
                            nc.tensor.matmul(
                                ps_out,
                                lhsT=ats[h][:, sbl * 128 : (sbl + 1) * 128],
                                rhs=woT_sb[:, h, e * 512 : (e + 1) * 512],
                                start=(h == 0),
                                stop=(h == QH_PER_CORE - 1),
                            )
                        # split evacuations between DVE and ACT (Copy is in
                        # the exp table set, so no table reload)
                        ev_eng = nc.vector.tensor_copy if e % 2 == 0 else nc.scalar.copy
                        ev_eng(o_sb[:, e * 512 : (e + 1) * 512], ps_out)
                    if not NO_OUTDMA:
                        nc.scalar.dma_start(
                            out=out_d[(4 * t + sbl) * 128 : (4 * t + sbl + 1) * 128, :],
                            in_=o_sb,
                        )

                prev_ats = None
                for t in range(NT if ("scores" in stages) else 0):
                    nkb = 4 * (t + 1)
                    at_tiles = []
                    for h in range(QH_PER_CORE):
                        qs = qt_sb[:, h, t * 512 : (t + 1) * 512]
                        ps_o = pbp.tile([128, 512], dt.float32, tag="attnT", bufs=1)
                        dacc = pb.tile([128, 512], dt.float32, tag="dacc", bufs=2)
                        for kb in range(0, nkb, 2):
                            ps_s = pbp.tile([128, 1024], dt.float32, tag="scores")
                            for j in (0, 1):
                                nc.tensor.matmul(
                                    ps_s[:, j * 512 : (j + 1) * 512],
                                    lhsT=kt_sb[:, (kb + j) * 128 : (kb + j + 1) * 128],
                                    rhs=qs,
                                    start=True,
                                    stop=True,
                                )
                            if PT_CONST:
                                pt = ptc_sb
                            else:
                                pt = pb.tile(
                                    [128, 1024], dt.bfloat16, tag="pt", bufs=4
                                )
                                if not NO_EXP:
                                    nc.scalar.activation(pt, ps_s, Exp, scale=SCALE)
                                else:
                                    nc.gpsimd.memset(pt, 0.5)
                                for j in (0, 1):
                                    if kb + j >= 4 * t:
                                        # masks run on the otherwise-idle POOL
                                        nc.gpsimd.tensor_mul(
                                            pt[:, j * 512 : (j + 1) * 512],
                                            pt[:, j * 512 : (j + 1) * 512],
                                            mask_sb[:, kb + j - 4 * t, :],
                                        )
                            if "pv" in stages:
                                for j in (0, 1):
                                    ptj = pt[:, j * 512 : (j + 1) * 512] if not PT_CONST else ptc_sb
                                    nc.tensor.matmul(
                                        ps_o,
                                        lhsT=v_sb[:, kb + j, :],
                                        rhs=ptj,
                                        start=(kb + j == 0),
                                        stop=(kb + j == nkb - 1),
                                    )
                                    # denominator partials accumulate on DVE
                                    if kb + j == 0:
                                        nc.vector.tensor_copy(dacc, ptj)
                                    else:
                                        nc.vector.tensor_add(dacc, dacc, ptj)
                        at = pb.tile([128, 512], dt.bfloat16, tag=f"at{h}")
                        if "pv" in stages:
                            # partition-reduce + broadcast the denominator in
                            # one fp32 matmul: every output row = column sum
                            ps_d = pbp.tile([128, 512], dt.float32, tag="denom", bufs=1)
                            nc.tensor.matmul(
                                ps_d, lhsT=ones_sb, rhs=dacc, start=True, stop=True
                            )
                            recip = pb.tile([128, 512], dt.float32, tag="recip")
                            nc.vector.reciprocal(recip, ps_d)
                            nc.vector.tensor_mul(at, ps_o, recip)
                        at_tiles.append(at)
                        if prev_ats is not None:
                            oproj_group(t - 1, h, prev_ats)
                    prev_ats = at_tiles
                if prev_ats is not None:
                    for sbl in range(4):
                        oproj_group(NT - 1, sbl, prev_ats)
    nc.compile()
    return nc


def _prep_inputs(x, cos, sin, wq, wk, wv, wo):
    x = np.asarray(x, dtype=np.float32)
    cos = np.asarray(cos, dtype=np.float32)
    sin = np.asarray(sin, dtype=np.float32)
    wq = np.asarray(wq, dtype=np.float32)
    wk = np.asarray(wk, dtype=np.float32)
    wv = np.asarray(wv, dtype=np.float32)
    wo = np.asarray(wo, dtype=np.float32)

    xt = np.ascontiguousarray(x[0].T).astype(bf16)  # [DIM, S]
    # cos/sin transposed and duplicated into both partition halves [128, S]
    csd = np.ascontiguousarray(np.tile(cos.T, (2, 1)).astype(np.float32))
    snd = np.ascontiguousarray(np.tile(sin.T, (2, 1)).astype(np.float32))
    # de-interleave perm: head dim pairs (2i, 2i+1) -> rows (i, 64+i)
    perm = np.concatenate([np.arange(0, HD, 2), np.arange(1, HD, 2)])

    # causal masks for the 4 diagonal sub-blocks: mask[r, c] = (r + delta) <= c
    r = np.arange(128)[:, None]
    c = np.arange(512)[None, :]
    mask = np.concatenate(
        [((r + d) <= c).astype(bf16) for d in (0, 128, 256, 384)], axis=0
    )  # [512, 512]

    in_maps = []
    for i in range(N_CORES):
        wq_i = wq[DQ * i : DQ * (i + 1)]  # [512, DIM]
        wk_i = wk[HD * i : HD * (i + 1)]  # [128, DIM]
        wv_i = wv[HD * i : HD * (i + 1)]
        wq_p = wq_i.reshape(QH_PER_CORE, HD, DIM)[:, perm, :].reshape(DQ, DIM)
        wk_p = wk_i[perm, :]
        wt = np.concatenate([wq_p.T, wk_p.T, wv_i.T], axis=1).astype(bf16)
        wot = np.ascontiguousarray(wo[:, DQ * i : DQ * (i + 1)].T).astype(
            bf16
        )  # [512, DIM]
        in_maps.append(
            {
                "xt": xt,
                "wt": np.ascontiguousarray(wt),
                "wot": wot,
                "csd": csd,
                "snd": snd,
                "mask": np.ascontiguousarray(mask),
            }
        )
    return in_maps


def _get_runner():
    global _RUNNER
    if _RUNNER is None:
        _RUNNER = _build()
    return _RUNNER


def kernel(x, cos, sin, wq, wk, wv, wo):
    from concourse.bass_utils import run_bass_kernel_spmd

    nc = _get_runner()
    in_maps = _prep_inputs(x, cos, sin, wq, wk, wv, wo)
    res = run_bass_kernel_spmd(nc, in_maps, list(range(N_CORES)))
    out = np.zeros((S, DIM), dtype=np.float32)
    for i in range(N_CORES):
        out += res.results[i]["out"]
    return out[None].astype(np.float32)



# revision 14
# speedup vs baseline: 163.1708x; 163.1708x over previous
"""Trainium2 Bass kernel for GQA attention prefill (Mistral-style, RoPE, causal).

B=1, S=2048, DIM=4096, 32 Q heads / 8 KV heads, HD=128, rope theta 1e6.

Sharding: tensor-parallel over heads across 8 cores. Core i gets Q heads
4i..4i+3 and KV head i. x is replicated (pre-transposed + fp16-cast on host).
Each core computes its 4 heads' attention and a partial output projection
(contraction over its 512 input dims of wo); the host sums the 8 partials.

Per-core dataflow (all matmuls fp16 with fp32 PSUM accumulation):

phase A (projections, dtile-major so the stationary weight tile is reused
across the full s=2048 moving range -> LDWEIGHTS amortized to ~3%):
  for dtile in [V, K, Q0..Q3]:
    psum[128, 2048] accumulated over 32 contraction blocks
      (per cb: one LDW + 4 x 512-wide matmuls)
    Q/K: rope applied on DVE into qt/kt [d, s] fp16 (partition-half pairing,
         sign-folded sin table -> 4 DVE passes)
    V: cast to fp16, then 16x DMA XBAR transpose into v_sb [s, d]
  x is resident in SBUF (16MB fp16), streamed in per-cb DMAs so the first
  dtile can start before the full load completes.

phase B (attention + oproj, chunk = 512 q columns, 2-head pairs so kt/v
stationary tiles serve 2 x 512-wide matmuls per LDW):
  per chunk t, head pair hp:
    for kb in 0..4(t+1):
      scores_T [k,q0|q1] one 2-bank psum tile; exp on ACT (scale folded);
      causal diag blocks masked by fp16 mask multiply on DVE (2x packed);
      dacc (softmax denominator partials) accumulated in fp16 on DVE (2x);
      PV accumulates attn_T [d, q0|q1] in psum
      (scores run one kb ahead of PV; o-proj matmul groups of the previous
       chunk are interleaved as PE filler under the ACT exp latency)
    denominator: ones-matmul partition-reduce of dacc -> reciprocal_approx_fast
    at = attn_T * recip -> fp16 [d, s] tiles
  o-proj per chunk: 4 s-blocks x 4 e-groups of 1024; at slice stationary,
  wo moving (1024 cycles per LDW); psum evacuated fp16, DMA per s-block.
Output is fp16 partials [S, DIM]; host sums 8 cores in fp32.
"""

import numpy as np
import ml_dtypes

S = 2048
DIM = 4096
HD = 128
N_CORES = 8
QH_PER_CORE = 4  # 512 q dims per core
DQ = QH_PER_CORE * HD  # 512
SCALE = 1.0 / float(np.sqrt(HD))
SB = S // 128  # 16 s blocks
CB = DIM // 128  # 32 contraction blocks
NT = S // 512  # 4 q chunks
NEG = 4  # oproj e-groups of 1024 (DIM / 1024)
EXP_BIAS = -9.0  # exp(s*scale - 9): keeps fp16 pt finite (max score ~18.7)

f16 = np.float16

_RUNNER = None


def _build(reps=None):
    import concourse.bass as bass
    import concourse.mybir as mybir
    import concourse.tile as tile
    from concourse import bacc
    from contextlib import nullcontext

    dt = mybir.dt
    Exp = mybir.ActivationFunctionType.Exp

    nc = bacc.Bacc(
        "TRN2", target_bir_lowering=False, debug=False, num_devices=N_CORES
    )

    xt_d = nc.dram_tensor("xt", [DIM, S], dt.float16, kind="ExternalInput").ap()
    # weights pre-laid-out host-side as [c=128, dtile*4096] so the per-dtile
    # DMA reads 8KB/partition contiguously
    wt_d = nc.dram_tensor("wt", [128, 6 * CB * 128], dt.float16, kind="ExternalInput").ap()
    wot_d = nc.dram_tensor("wot", [DQ, DIM], dt.float16, kind="ExternalInput").ap()
    csd_d = nc.dram_tensor("csd", [128, S], dt.float16, kind="ExternalInput").ap()
    snd_d = nc.dram_tensor("snd", [128, S], dt.float16, kind="ExternalInput").ap()
    mask_d = nc.dram_tensor("mask", [512, 512], dt.float16, kind="ExternalInput").ap()
    out_d = nc.dram_tensor("out", [S, DIM], dt.float16, kind="ExternalOutput").ap()

    with tile.TileContext(nc) as tc:
        with tc.For_i(0, reps, 1) if reps else nullcontext(), tc.tile_pool(
            name="const", bufs=1
        ) as cp:
            mask_sb = cp.tile([128, 4, 512], dt.float16)
            nc.sync.dma_start(out=mask_sb, in_=mask_d.rearrange("(m p) n -> p m n", p=128))
            ones_sb = cp.tile([128, 128], dt.float16)
            nc.vector.memset(ones_sb, 1.0)
            ebias_sb = cp.tile([128, 1], dt.float32)
            nc.vector.memset(ebias_sb, EXP_BIAS)

            qt_sb = cp.tile([128, QH_PER_CORE, S], dt.float16)  # [d, h, s]
            kt_sb = cp.tile([128, S], dt.float16)  # [d, s]
            v_sb = cp.tile([128, SB, HD], dt.float16)  # [s128, sb, d]

            # ---------------- phase A: projections + rope ----------------
            with (
                tc.tile_pool(name="pa", bufs=1) as pa,
                tc.tile_pool(name="pap", bufs=2, space="PSUM") as pap,
            ):
                # small tables + the resident x stream go on the scalar DMA
                # queue; weight tiles on the sync queue, so the first matmul
                # only waits for wt[Q0] + xt[cb0] (parallel queues).
                csd_sb = pa.tile([128, S], dt.float16)
                nc.scalar.dma_start(out=csd_sb, in_=csd_d)
                snd_sb = pa.tile([128, S], dt.float16)  # sign-folded: -sin | +sin
                nc.scalar.dma_start(out=snd_sb, in_=snd_d)

                xt_sb = pa.tile([128, CB, S], dt.float16)
                xt_r = xt_d.rearrange("(cb c) s -> c cb s", c=128)
                for cb in range(CB):
                    nc.scalar.dma_start(out=xt_sb[:, cb, :], in_=xt_r[:, cb, :])

                def rope_evac(ps, dest):
                    # dest[0:64]   = a*cos - b*sin   (a = rows 0:64, b = rows 64:128)
                    # dest[64:128] = a*sin + b*cos
                    tb = pa.tile([128, S], dt.float16, tag="tb")
                    nc.vector.tensor_mul(dest, ps, csd_sb)
                    nc.vector.tensor_mul(tb[0:64, :], ps[64:128, :], snd_sb[0:64, :])
                    nc.vector.tensor_mul(tb[64:128, :], ps[0:64, :], snd_sb[64:128, :])
                    nc.vector.tensor_add(dest, dest, tb)

                # processing order: Q0..Q3, K, V — V last so the phase A/B
                # psum-pool boundary only waits on V's cheap cast (not a rope)
                for dtile in (0, 1, 2, 3, 4, 5):
                    wt_sb = pa.tile([128, CB, 128], dt.float16, tag="wt", bufs=2)
                    nc.sync.dma_start(
                        out=wt_sb,
                        in_=wt_d[:, dtile * 4096 : (dtile + 1) * 4096].rearrange(
                            "c (cb n) -> c cb n", n=128
                        ),
                    )
                    ps = pap.tile([128, S], dt.float32, tag="proj")
                    for cb in range(CB):
                        for j in range(4):
                            nc.tensor.matmul(
                                ps[:, j * 512 : (j + 1) * 512],
                                lhsT=wt_sb[:, cb, :],
                                rhs=xt_sb[:, cb, j * 512 : (j + 1) * 512],
                                start=(cb == 0),
                                stop=(cb == CB - 1),
                            )
                    if dtile < QH_PER_CORE:
                        rope_evac(ps, qt_sb[:, dtile, :])
                    elif dtile == 4:
                        rope_evac(ps, kt_sb)
                    elif dtile == 5:
                        vt_st = pa.tile([128, S], dt.float16, tag="vt")
                        nc.vector.tensor_copy(vt_st, ps)
                        for b in range(SB):
                            nc.sync.dma_start_transpose(
                                out=v_sb[:, b, :],
                                in_=vt_st[:, b * 128 : (b + 1) * 128],
                            )

            # ---------------- phase B: attention + output projection ------
            with (
                tc.tile_pool(name="pb", bufs=2) as pb,
                tc.tile_pool(name="pbp", bufs=1, space="PSUM") as pbp,
            ):
                woT_sb = pb.tile([128, QH_PER_CORE, DIM], dt.float16, bufs=1)
                woT_r = wot_d.rearrange("(db p) e -> p db e", p=128)
                for eh in range(4):  # split so early oproj e-groups start sooner
                    nc.sync.dma_start(
                        out=woT_sb[:, :, eh * 1024 : (eh + 1) * 1024],
                        in_=woT_r[:, :, eh * 1024 : (eh + 1) * 1024],
                    )

                def oproj_group(t, sbl, eg, ats):
                    # out rows (4t+sbl)*128, e-columns eg*1024..+1024
                    ps_op = pbp.tile([128, 1024], dt.float32, tag="oproj", bufs=1)
                    for hp in (0, 1):
                        for hh in (0, 1):
                            h = 2 * hp + hh
                            lhsT = ats[hp][
                                :, hh * 512 + sbl * 128 : hh * 512 + (sbl + 1) * 128
                            ]
                            for j in (0, 1):
                                nc.tensor.matmul(
                                    ps_op[:, j * 512 : (j + 1) * 512],
                                    lhsT=lhsT,
                                    rhs=woT_sb[
                                        :, h, eg * 1024 + j * 512 : eg * 1024 + (j + 1) * 512
                                    ],
                                    start=(h == 0),
                                    stop=(h == 3),
                                )
                    o_sb = osb_tiles[sbl]
                    ev = nc.vector.tensor_copy if eg % 2 == 0 else nc.scalar.copy
                    ev(o_sb[:, eg * 1024 : (eg + 1) * 1024], ps_op)
                    if eg == NEG - 1:
                        nc.scalar.dma_start(
                            out=out_d[(4 * t + sbl) * 128 : (4 * t + sbl + 1) * 128, :],
                            in_=o_sb,
                        )

                pending = []  # oproj closures for the previous chunk
                prev_ats = None
                for t in range(NT):
                    nkb = 4 * (t + 1)
                    if prev_ats is not None:
                        pats = prev_ats
                        pt_ = t - 1
                        pending = [
                            (pt_, sbl, eg, pats)
                            for sbl in range(4)
                            for eg in range(NEG)
                        ]
                        osb_tiles = [
                            pb.tile([128, DIM], dt.float16, name=f"osb_{t}_{sbl}",
                                    tag=f"osb{sbl}", bufs=1)
                            for sbl in range(4)
                        ]
                    at_tiles = []
                    for hp in (0, 1):
                        h0, h1 = 2 * hp, 2 * hp + 1
                        q0 = qt_sb[:, h0, t * 512 : (t + 1) * 512]
                        q1 = qt_sb[:, h1, t * 512 : (t + 1) * 512]
                        ps_o = pbp.tile([128, 1024], dt.float32, tag="pvo", bufs=1)
                        dacc = pb.tile([128, 1024], dt.float16, tag="dacc", bufs=2)
                        pt_tiles = [None] * nkb

                        def emit_scores(kb):
                            ps_s = pbp.tile(
                                [128, 1024], dt.float32, tag="scores", bufs=2
                            )
                            nc.tensor.matmul(
                                ps_s[:, 0:512],
                                lhsT=kt_sb[:, kb * 128 : (kb + 1) * 128],
                                rhs=q0, start=True, stop=True,
                            )
                            nc.tensor.matmul(
                                ps_s[:, 512:1024],
                                lhsT=kt_sb[:, kb * 128 : (kb + 1) * 128],
                                rhs=q1, start=True, stop=True,
                            )
                            pt = pb.tile([128, 1024], dt.float16, tag="pt", bufs=4)
                            nc.scalar.activation(pt, ps_s, Exp, scale=SCALE, bias=ebias_sb)
                            pt_tiles[kb] = pt

                        def emit_mask_dacc(kb):
                            pt = pt_tiles[kb]
                            if kb >= 4 * t:  # diagonal block: causal mask
                                m = mask_sb[:, kb - 4 * t, :]
                                nc.vector.tensor_mul(pt[:, 0:512], pt[:, 0:512], m)
                                nc.vector.tensor_mul(pt[:, 512:1024], pt[:, 512:1024], m)
                            if kb == 0:
                                nc.vector.tensor_copy(dacc, pt)
                            else:
                                nc.vector.tensor_add(dacc, dacc, pt)

                        emit_scores(0)
                        for kb in range(nkb):
                            if kb + 1 < nkb:
                                emit_scores(kb + 1)
                            emit_mask_dacc(kb)
                            pt = pt_tiles[kb]
                            # 2 oproj groups at the hp boundary to cover the
                            # denominator->at chain of the previous hp
                            for _ in range(2 if kb == 0 else 1):
                                if pending:
                                    oproj_group(*pending.pop(0))
                            for j in (0, 1):
                                nc.tensor.matmul(
                                    ps_o[:, j * 512 : (j + 1) * 512],
                                    lhsT=v_sb[:, kb, :],
                                    rhs=pt[:, j * 512 : (j + 1) * 512],
                                    start=(kb == 0),
                                    stop=(kb == nkb - 1),
                                )
                        # softmax denominator: every row of ps_d = column sum
                        ps_d = pbp.tile([128, 1024], dt.float32, tag="scores", bufs=2)
                        for j in (0, 1):
                            nc.tensor.matmul(
                                ps_d[:, j * 512 : (j + 1) * 512],
                                lhsT=ones_sb,
                                rhs=dacc[:, j * 512 : (j + 1) * 512],
                                start=True, stop=True,
                            )
                        recip = pb.tile([128, 1024], dt.float32, tag="recip")
                        nc.vector.reciprocal_approx_fast(recip, ps_d)
                        at = pb.tile([128, 1024], dt.float16, tag=f"at{hp}", bufs=2)
                        nc.vector.tensor_mul(at, ps_o, recip)
                        at_tiles.append(at)
                    while pending:
                        oproj_group(*pending.pop(0))
                    prev_ats = at_tiles
                # flush last chunk's oproj
                osb_tiles = [
                    pb.tile([128, DIM], dt.float16, name=f"osb_f_{sbl}",
                            tag=f"osb{sbl}", bufs=1)
                    for sbl in range(4)
                ]
                for sbl in range(4):
                    for eg in range(NEG):
                        oproj_group(NT - 1, sbl, eg, prev_ats)
    nc.compile()
    return nc


def _prep_inputs(x, cos, sin, wq, wk, wv, wo):
    x = np.asarray(x, dtype=np.float32)
    cos = np.asarray(cos, dtype=np.float32)
    sin = np.asarray(sin, dtype=np.float32)
    wq = np.asarray(wq, dtype=np.float32)
    wk = np.asarray(wk, dtype=np.float32)
    wv = np.asarray(wv, dtype=np.float32)
    wo = np.asarray(wo, dtype=np.float32)

    xt = np.ascontiguousarray(x[0].T).astype(f16)  # [DIM, S]
    # cos/sin transposed, duplicated into both partition halves [128, S];
    # sin sign-folded: rows 0:64 = -sin (pairs with b), rows 64:128 = +sin
    csd = np.tile(cos.T, (2, 1)).astype(f16)
    snd = np.concatenate([-sin.T, sin.T], axis=0).astype(f16)
    # de-interleave perm: head dim pairs (2i, 2i+1) -> rows (i, 64+i)
    perm = np.concatenate([np.arange(0, HD, 2), np.arange(1, HD, 2)])

    # causal masks for the 4 diagonal sub-blocks: mask[r, c] = (r + delta) <= c
    r = np.arange(128)[:, None]
    c = np.arange(512)[None, :]
    mask = np.concatenate(
        [((r + d) <= c).astype(f16) for d in (0, 128, 256, 384)], axis=0
    )  # [512, 512]

    in_maps = []
    for i in range(N_CORES):
        wq_i = wq[DQ * i : DQ * (i + 1)]  # [512, DIM]
        wk_i = wk[HD * i : HD * (i + 1)]  # [128, DIM]
        wv_i = wv[HD * i : HD * (i + 1)]
        wq_p = wq_i.reshape(QH_PER_CORE, HD, DIM)[:, perm, :].reshape(DQ, DIM)
        wk_p = wk_i[perm, :]
        w_all = np.concatenate([wq_p, wk_p, wv_i], axis=0)  # [768, DIM]
        # layout [c=128, dtile, cb, n=128]: element (dtile*128+n, cb*128+c)
        wt = (
            w_all.reshape(6, 128, CB, 128)  # [dtile, n, cb, c]
            .transpose(3, 0, 2, 1)  # [c, dtile, cb, n]
            .reshape(128, 6 * CB * 128)
            .astype(f16)
        )
        wot = np.ascontiguousarray(wo[:, DQ * i : DQ * (i + 1)].T).astype(f16)
        in_maps.append(
            {
                "xt": xt,
                "wt": np.ascontiguousarray(wt),
                "wot": wot,
                "csd": np.ascontiguousarray(csd),
                "snd": np.ascontiguousarray(snd),
                "mask": np.ascontiguousarray(mask),
            }
        )
    return in_maps


def _get_runner():
    global _RUNNER
    if _RUNNER is None:
        _RUNNER = _build()
    return _RUNNER


def kernel(x, cos, sin, wq, wk, wv, wo):
    from concourse.bass_utils import run_bass_kernel_spmd

    nc = _get_runner()
    in_maps = _prep_inputs(x, cos, sin, wq, wk, wv, wo)
    res = run_bass_kernel_spmd(nc, in_maps, list(range(N_CORES)))
    out = np.zeros((S, DIM), dtype=np.float32)
    for i in range(N_CORES):
        out += res.results[i]["out"].astype(np.float32)
    return out[None].astype(np.float32)
